# revision 18
# baseline (speedup 1.0000x reference)
"""Trainium2 Bass kernel for nn_CrossAttention (self-attention, B=2, N=4096,
QD=512, 8 heads x 64 dim).

Sharding: 16 (batch, head) pairs across 8 cores -> core c handles batch c//4
and heads {2*(c%4), 2*(c%4)+1}.  Projection weights are column-sliced (Wq/Wk/Wv)
and row-sliced (Wo) per core; each core emits a partial [4096, 512] output that
the host sums per batch (row-parallel Wo => all-reduce done on host at gather).

ScalarE exp is the bottleneck engine; the kernel is built to keep it >95% busy
and to minimize per-ACTIVATE overhead (~280ns fixed cost per instruction):

  - Flat software pipeline over all 256 j-tiles (slice boundaries do not
    serialize: next slice's QK^T is emitted before this slice's AV drain).
  - Per j-tile: row-tiled QK^T pair (K=64 heads at PE row groups 0/64) ->
    S^T [128j, 1024] fp32 in a 2-bank PSUM group.
  - exp staging: within each octet of j-tiles, tiles 0-6 are copied
    PSUM->SBUF by the otherwise-idle GPSIMD (pos 0,2,4,6) and DVE (1,3,5),
    then exp'd in two large ACTIVATEs ([128,4096] + [128,3072], ~890-930ns
    per tile vs ~1130 direct); tile 7 is exp'd straight from PSUM.  Slice 0
    is all-direct (PE-bound there due to k/v/V' production).
  - V' carries a ones column so softmax denominators fall out of the AV
    matmul (row 64).  AV matmuls (M=65, both heads) lag QK by LAG=6 tiles.
  - Epilogue per slice, fully off the critical path: den rows are moved onto
    partitions with eight K=1 PE matmuls -> one [128,8] reciprocal (~0.2us,
    replaces two 3.3us single-partition iterative divides) -> PE transpose ->
    PE broadcast matmuls -> normalize muls -> Wo -> DMA.  No DVE op in the
    steady state exceeds ~0.7us, so the strict-FIFO queues never head-block.
  - Tile-scheduler virtual clock (tile_set_cur_wait) pins every iteration to
    its real-time slot; without it the scheduler hoists future slices' work
    into earlier queue positions and stalls the pipeline at slice boundaries.
"""

import sys

sys.path.insert(0, "/opt/trn_rl_repo")

import numpy as np
import ml_dtypes

import concourse.bass as bass
import concourse.mybir as mybir
from concourse import bacc
from concourse.tile import TileContext
from concourse.bass_utils import run_bass_kernel_spmd
from concourse.masks import make_identity

B, N, QD = 2, 4096, 512
HEADS, DIM_HEAD = 8, 64
INNER = HEADS * DIM_HEAD
SCALE = DIM_HEAD**-0.5

NCORES = 8
HPC = 2  # heads per core
D2 = HPC * DIM_HEAD  # 128
KT = 4  # k tiles of 128 over QD=512
ISL = 512  # i slice
NI = N // ISL  # 8
JTL = 128  # j tile
NJ = N // JTL  # 32
LAG = 4  # AV matmuls trail QK/exp by this many j-tiles
TOT = NI * NJ  # 256

F32 = mybir.dt.float32
BF16 = mybir.dt.bfloat16
BFNP = ml_dtypes.bfloat16
EXP = mybir.ActivationFunctionType.Exp

# exp staging pattern within each octet of j-tiles (slices >= 1): pos 0-3
# copied PSUM->SBUF by the DVE, exp'd in one [128,4096] ACTIVATE; pos 4-7
# exp'd straight from PSUM.  (GPSIMD cannot access PSUM, so the DVE is the
# only spare stager; 4/8 staged keeps it ~7us/slice under the ACT pace.)


def build_program():
    nc = bacc.Bacc("TRN2", target_bir_lowering=False, debug=False,
                   num_devices=NCORES)

    xT = nc.dram_tensor("xT", [QD, N], BF16, kind="ExternalInput").ap()
    wq = nc.dram_tensor("wq", [QD, D2], BF16, kind="ExternalInput").ap()
    wk = nc.dram_tensor("wk", [QD, D2], BF16, kind="ExternalInput").ap()
    wv = nc.dram_tensor("wv", [QD, D2], BF16, kind="ExternalInput").ap()
    wo = nc.dram_tensor("wo", [D2, QD], BF16, kind="ExternalInput").ap()
    bsel_d = nc.dram_tensor("bsel", [8, 4 * 128], BF16, kind="ExternalInput").ap()
    out = nc.dram_tensor("out", [N, QD], F32, kind="ExternalOutput").ap()

    with TileContext(nc) as tc:
        with tc.tile_pool(name="persist", bufs=1) as pp, \
             tc.tile_pool(name="st_ps", bufs=2, space="PSUM") as st_ps, \
             tc.tile_pool(name="av_ps", bufs=1, space="PSUM") as av_ps, \
             tc.tile_pool(name="aux_ps", bufs=1, space="PSUM") as aux_ps, \
             tc.tile_pool(name="p0_sb", bufs=8) as p0_sb, \
             tc.tile_pool(name="n_sb", bufs=2) as n_sb:
            x_sb = pp.tile([128, KT, N], BF16)
            wq_sb = pp.tile([128, KT, D2], BF16)
            wk_sb = pp.tile([128, KT, D2], BF16)
            wv_sb = pp.tile([128, KT, D2], BF16)
            wo_sb = pp.tile([128, QD], BF16)
            ident = pp.tile([128, 128], BF16)
            identF = pp.tile([128, 128], F32)
            qT = pp.tile([128, N], BF16)
            kT = pp.tile([128, N], BF16)
            vT = pp.tile([128, N], BF16)
            v0p = pp.tile([128, NJ, DIM_HEAD + 1], BF16)
            v1p = pp.tile([128, NJ, DIM_HEAD + 1], BF16)
            ones_col = pp.tile([128, 1], F32)
            ones_bf = pp.tile([128, 1], BF16)
            bsel = pp.tile([8, 4 * 128], BF16)
            nc.sync.dma_start(out=bsel[:], in_=bsel_d[:])

            xTr = xT.rearrange("(k p) n -> p k n", p=128)
            # x slice 0 + wk + wq gate the first QK^T: issue them first.
            nc.sync.dma_start(out=x_sb[:, :, 0:ISL], in_=xTr[:, :, 0:ISL])
            nc.sync.dma_start(out=wk_sb[:], in_=wk.rearrange("(k p) m -> p k m", p=128))
            nc.sync.dma_start(out=wq_sb[:], in_=wq.rearrange("(k p) m -> p k m", p=128))
            nc.sync.dma_start(out=wv_sb[:], in_=wv.rearrange("(k p) m -> p k m", p=128))
            # spread x-slice loads over three engine DMA queues so the
            # transfers overlap (the sync queue alone serializes at ~2-4us
            # per slice and paces slice 0's prologue otherwise)
            for s in range(1, NI):
                ssl = slice(s * ISL, (s + 1) * ISL)
                eng = (nc.sync, nc.gpsimd, nc.scalar)[s % 3]
                eng.dma_start(out=x_sb[:, :, ssl], in_=xTr[:, :, ssl])
            nc.sync.dma_start(out=wo_sb[:], in_=wo[:])
            make_identity(nc, ident[:])
            make_identity(nc, identF[:])
            nc.gpsimd.memset(v0p[:, :, DIM_HEAD], 1.0)
            nc.gpsimd.memset(v1p[:, :, DIM_HEAD], 1.0)
            nc.gpsimd.memset(ones_col[:], 1.0)
            nc.gpsimd.memset(ones_bf[:], 1.0)

            def proj(w_sb, dst, s, tag="ps"):
                """dst[:, s*ISL:(s+1)*ISL] = (W^T @ x^T) slice via psum."""
                ssl = slice(s * ISL, (s + 1) * ISL)
                ps = aux_ps.tile([128, ISL], F32, tag=tag, name="projps") if tag != "st" \
                    else st_ps.tile([128, ISL], F32, tag="st", name="projst")
                for k in range(KT):
                    nc.tensor.matmul(ps[:], w_sb[:, k, :], x_sb[:, k, ssl],
                                     start=(k == 0), stop=(k == KT - 1))
                nc.vector.tensor_copy(out=dst[:, ssl], in_=ps[:])

            def transp(j):
                """V'[j] tiles from vT via PE transpose (both heads)."""
                tp = aux_ps.tile([128, 128], BF16, tag="aux", name="tp")
                nc.tensor.transpose(tp[:], vT[:, j * JTL:(j + 1) * JTL], ident[:])
                nc.vector.tensor_copy(out=v0p[:, j, 0:DIM_HEAD], in_=tp[:, 0:DIM_HEAD])
                nc.vector.tensor_copy(out=v1p[:, j, 0:DIM_HEAD], in_=tp[:, DIM_HEAD:D2])

            states = {}

            def epi(i_prev, step, tail=False):
                """Deferred epilogue for slice i_prev (runs during i_prev+1)."""
                e = states[i_prev]
                if step == 0:  # av -> SBUF (frees av PSUM; source for den/lh)
                    e["av_sb0"] = n_sb.tile([DIM_HEAD + 1, ISL], F32, tag="av_sb0", name="av_sb0")
                    e["av_sb1"] = n_sb.tile([DIM_HEAD + 1, ISL], F32, tag="av_sb1", name="av_sb1")
                    nc.vector.tensor_copy(out=e["av_sb0"][:], in_=e["av0"][:])
                    nc.vector.tensor_copy(out=e["av_sb1"][:], in_=e["av1"][:])
                elif step == 1:  # den rows to bf16 (one partition each)
                    e["db0"] = n_sb.tile([1, ISL], BF16, tag="db0", name="db0")
                    e["db1"] = n_sb.tile([1, ISL], BF16, tag="db1", name="db1")
                    nc.vector.tensor_copy(out=e["db0"][:],
                                          in_=e["av_sb0"][DIM_HEAD:DIM_HEAD + 1, :])
                    nc.vector.tensor_copy(out=e["db1"][:],
                                          in_=e["av_sb1"][DIM_HEAD:DIM_HEAD + 1, :])
                elif step == 11:  # den rows -> partitions via eight K=1 matmuls
                    e["dnt"] = aux_ps.tile([128, 8], F32, tag="ps", name="dnt")
                    for s in range(4):
                        for h in range(2):
                            db = e["db0"] if h == 0 else e["db1"]
                            c = 2 * s + h
                            nc.tensor.matmul(
                                e["dnt"][:, c:c + 1],
                                db[0:1, s * 128:(s + 1) * 128],
                                ones_bf[0:1, 0:1],
                                start=True, stop=True)
                elif step == 2:  # one wide reciprocal (0.2us vs 2x 3.3us)
                    e["rT"] = n_sb.tile([128, 8], F32, tag="rT", name="rT")
                    nc.vector.reciprocal(e["rT"][:], e["dnt"][:])
                elif step == 3:  # transpose rT back: [8, 128] = rT^T
                    e["rtt_ps"] = aux_ps.tile([8, 128], F32, tag="ps", name="rtt_ps")
                    nc.tensor.matmul(e["rtt_ps"][:], e["rT"][:], identF[:],
                                     start=True, stop=True)
                elif step == 4:
                    e["rtt"] = n_sb.tile([8, 128], BF16, tag="rtt", name="rtt")
                    nc.vector.tensor_copy(out=e["rtt"][:], in_=e["rtt_ps"][:])
                elif step == 5:  # broadcast recips along d2 via 4 PE matmuls
                    e["rb"] = aux_ps.tile([128, ISL], F32, tag="aux", name="rb")
                    for s in range(4):
                        nc.tensor.matmul(e["rb"][:, s * 128:(s + 1) * 128],
                                         bsel[:, s * 128:(s + 1) * 128],
                                         e["rtt"][:], start=True, stop=True)
                elif step == 6:  # normalize -> lh (bf16)
                    e["lh"] = n_sb.tile([128, ISL], BF16, tag="lh", name="lh")
                    nc.vector.tensor_mul(out=e["lh"][0:64, :],
                                         in0=e["av_sb0"][0:DIM_HEAD, :],
                                         in1=e["rb"][0:64, :])
                    nc.vector.tensor_mul(out=e["lh"][64:128, :],
                                         in0=e["av_sb1"][0:DIM_HEAD, :],
                                         in1=e["rb"][64:128, :])
                else:  # steps 7..10: one Wo matmul + store each
                    s = step - 7
                    if tail:  # st pool is idle at the tail: wider ladder
                        wop = st_ps.tile([128, QD], F32, tag="st", name=f"wot{s}")
                    else:
                        wop = aux_ps.tile([128, QD], F32, tag="aux", name="wop")
                    nc.tensor.matmul(wop[:], e["lh"][:, s * 128:(s + 1) * 128],
                                     wo_sb[:], start=True, stop=True)
                    wos = n_sb.tile([128, QD], F32, tag="wos", name="wos", bufs=4)
                    nc.vector.tensor_copy(out=wos[:], in_=wop[:])
                    nc.sync.dma_start(
                        out=out[i_prev * ISL + s * 128:i_prev * ISL + (s + 1) * 128, :],
                        in_=wos[:])

            # epilogue emission slots (j within the following slice)
            EPI = {6: 0, 7: 1, 8: 11, 9: 2, 12: 3, 13: 4, 14: 5, 15: 6,
                   16: 7, 18: 8, 20: 9, 22: 10}

            # virtual-clock pacing (see module docstring)
            HEAD_US = 13.0
            PACE0_US = 1.8
            PACE_US = 1.16

            def slot_ms(g):
                if g < NJ:
                    return (HEAD_US + g * PACE0_US) / 1e3
                return (HEAD_US + NJ * PACE0_US + (g - NJ) * PACE_US) / 1e3

            # warm-up: k and q projections for slice 0 (independent st slots)
            proj(wk_sb, kT, 0, tag="st")
            proj(wq_sb, qT, 0, tag="st")

            octs = {}  # (i, o) -> dict with sga/sgb/pta/ptb/ptu tiles

            def p_src(ga):
                """(ap, col0) holding exp'd tile ga for the AV matmuls."""
                return octs[ga], 0

            for g in range(TOT + LAG):
                tc.tile_set_cur_wait(ms=slot_ms(g))
                i, j = divmod(g, NJ) if g < TOT else (NI, g - TOT)
                if g < TOT:
                    if j == 0:
                        states[i] = {
                            "av0": av_ps.tile([DIM_HEAD + 1, ISL], F32, tag="av0", name="av0"),
                            "av1": av_ps.tile([DIM_HEAD + 1, ISL], F32, tag="av1", name="av1"),
                        }
                    isl = slice(i * ISL, (i + 1) * ISL)
                    jsl = slice(j * JTL, (j + 1) * JTL)
                    st = st_ps.tile([128, 2 * ISL], F32, tag="st", name="st")
                    nc.tensor.matmul(st[:, 0:ISL], kT[0:64, jsl], qT[0:64, isl],
                                     start=True, stop=True)
                    nc.tensor.matmul(st[:, ISL:2 * ISL], kT[64:128, jsl],
                                     qT[64:128, isl], start=True, stop=True)
                    pt = p0_sb.tile([128, 2 * ISL], BF16, tag="pt0", name="pt0")
                    nc.scalar.activation(pt[:], st[:], EXP, scale=SCALE)
                    octs[g] = pt
                if g >= LAG:
                    ga = g - LAG
                    ia, ja = divmod(ga, NJ)
                    src, c0 = p_src(ga)
                    eia = states[ia]
                    nc.tensor.matmul(eia["av0"][:], v0p[:, ja, :],
                                     src[:, c0:c0 + ISL],
                                     start=(ja == 0), stop=(ja == NJ - 1))
                    nc.tensor.matmul(eia["av1"][:], v1p[:, ja, :],
                                     src[:, c0 + ISL:c0 + 2 * ISL],
                                     start=(ja == 0), stop=(ja == NJ - 1))
                # slice-0 prologue: stream k/v/V' production
                if i == 0:
                    if j == 0:
                        proj(wv_sb, vT, 0)
                    elif j == 1:
                        proj(wk_sb, kT, 1)
                    elif j == 2:
                        proj(wv_sb, vT, 1)
                    elif j == 3:
                        for jj in range(4):
                            transp(jj)
                    elif j == 4:
                        for jj in range(4, 8):
                            transp(jj)
                    elif j >= 5 and j % 4 in (1, 2, 3):
                        s = j // 4 + 1
                        if s < NI:
                            if j % 4 == 1:
                                proj(wk_sb, kT, s)
                            elif j % 4 == 2:
                                proj(wv_sb, vT, s)
                            elif j > 5:
                                for jj in range(4 * s, 4 * s + 4):
                                    transp(jj)
                # deferred epilogue of slice i-1
                if 1 <= i < NI and j in EPI:
                    epi(i - 1, EPI[j])
                # next slice's q projection
                if g < TOT and j == 10 and i + 1 < NI:
                    proj(wq_sb, qT, i + 1)

            # tail: full epilogue for the last slice
            for sidx, step in enumerate((0, 1, 11, 2, 3, 4, 5, 6, 7, 8, 9, 10)):
                tc.tile_set_cur_wait(ms=slot_ms(TOT + LAG) + 0.0002 * sidx)
                epi(NI - 1, step, tail=True)

    nc.compile()
    return nc


_NC = None


def _get_program():
    global _NC
    if _NC is None:
        _NC = build_program()
    return _NC


def _bsel_host():
    b = np.zeros((8, 512), dtype=np.float32)
    for s in range(4):
        b[2 * s, s * 128:s * 128 + 64] = 1.0
        b[2 * s + 1, s * 128 + 64:(s + 1) * 128] = 1.0
    return b.astype(BFNP)


def kernel(x, Wq, Wk, Wv, Wo, bo):
    x = np.asarray(x, dtype=np.float32)
    Wq = np.asarray(Wq, dtype=np.float32)
    Wk = np.asarray(Wk, dtype=np.float32)
    Wv = np.asarray(Wv, dtype=np.float32)
    Wo = np.asarray(Wo, dtype=np.float32)
    bo = np.asarray(bo, dtype=np.float32)

    nc = _get_program()

    in_maps = []
    for c in range(NCORES):
        b, m = divmod(c, NCORES // B)
        cs = slice(m * D2, (m + 1) * D2)
        in_maps.append({
            "xT": np.ascontiguousarray(x[b].T).astype(BFNP),
            "wq": np.ascontiguousarray(Wq[:, cs]).astype(BFNP),
            "wk": np.ascontiguousarray(Wk[:, cs]).astype(BFNP),
            "wv": np.ascontiguousarray(Wv[:, cs]).astype(BFNP),
            "wo": np.ascontiguousarray(Wo[cs, :]).astype(BFNP),
            "bsel": _bsel_host(),
        })

    res = run_bass_kernel_spmd(nc, in_maps, core_ids=list(range(NCORES)))

    out = np.zeros((B, N, QD), dtype=np.float32)
    for c in range(NCORES):
        b = c // (NCORES // B)
        out[b] += res.results[c]["out"]
    out += bo[None, None, :]
    return out


# revision 19
# speedup vs baseline: 1.0404x; 1.0404x over previous
"""Trainium2 Bass kernel for nn_CrossAttention (self-attention, B=2, N=4096,
QD=512, 8 heads x 64 dim).

Sharding: 16 (batch, head) pairs across 8 cores -> core c handles batch c//4
and heads {2*(c%4), 2*(c%4)+1}.  Projection weights are column-sliced (Wq/Wk/Wv)
and row-sliced (Wo) per core; each core emits a partial [4096, 512] output that
the host sums per batch (row-parallel Wo => all-reduce done on host at gather).

ScalarE exp is the bottleneck engine; the kernel is built to keep it >95% busy
and to minimize per-ACTIVATE overhead (~280ns fixed cost per instruction):

  - Flat software pipeline over all 256 j-tiles (slice boundaries do not
    serialize: next slice's QK^T is emitted before this slice's AV drain).
  - Per j-tile: row-tiled QK^T pair (K=64 heads at PE row groups 0/64) ->
    S^T [128j, 1024] fp32 in a 2-bank PSUM group.
  - exp staging: within each octet of j-tiles, tiles 0-6 are copied
    PSUM->SBUF by the otherwise-idle GPSIMD (pos 0,2,4,6) and DVE (1,3,5),
    then exp'd in two large ACTIVATEs ([128,4096] + [128,3072], ~890-930ns
    per tile vs ~1130 direct); tile 7 is exp'd straight from PSUM.  Slice 0
    is all-direct (PE-bound there due to k/v/V' production).
  - V' carries a ones column so softmax denominators fall out of the AV
    matmul (row 64).  AV matmuls (M=65, both heads) lag QK by LAG=6 tiles.
  - Epilogue per slice, fully off the critical path: den rows are moved onto
    partitions with eight K=1 PE matmuls -> one [128,8] reciprocal (~0.2us,
    replaces two 3.3us single-partition iterative divides) -> PE transpose ->
    PE broadcast matmuls -> normalize muls -> Wo -> DMA.  No DVE op in the
    steady state exceeds ~0.7us, so the strict-FIFO queues never head-block.
  - Tile-scheduler virtual clock (tile_set_cur_wait) pins every iteration to
    its real-time slot; without it the scheduler hoists future slices' work
    into earlier queue positions and stalls the pipeline at slice boundaries.
"""

import sys

sys.path.insert(0, "/opt/trn_rl_repo")

import numpy as np
import ml_dtypes

import concourse.bass as bass
import concourse.mybir as mybir
from concourse import bacc
from concourse.tile import TileContext
from concourse.bass_utils import run_bass_kernel_spmd
from concourse.masks import make_identity

B, N, QD = 2, 4096, 512
HEADS, DIM_HEAD = 8, 64
INNER = HEADS * DIM_HEAD
SCALE = DIM_HEAD**-0.5

NCORES = 8
HPC = 2  # heads per core
D2 = HPC * DIM_HEAD  # 128
KT = 4  # k tiles of 128 over QD=512
ISL = 512  # i slice
NI = N // ISL  # 8
JTL = 128  # j tile
NJ = N // JTL  # 32
LAG = 4  # AV matmuls trail QK/exp by this many j-tiles
TOT = NI * NJ  # 256

F32 = mybir.dt.float32
BF16 = mybir.dt.bfloat16
BFNP = ml_dtypes.bfloat16
EXP = mybir.ActivationFunctionType.Exp

# exp staging pattern within each octet of j-tiles (slices >= 1): pos 0-3
# copied PSUM->SBUF by the DVE, exp'd in one [128,4096] ACTIVATE; pos 4-7
# exp'd straight from PSUM.  (GPSIMD cannot access PSUM, so the DVE is the
# only spare stager; 4/8 staged keeps it ~7us/slice under the ACT pace.)


def build_program():
    nc = bacc.Bacc("TRN2", target_bir_lowering=False, debug=False,
                   num_devices=NCORES)

    xT = nc.dram_tensor("xT", [QD, N], BF16, kind="ExternalInput").ap()
    wq = nc.dram_tensor("wq", [QD, D2], BF16, kind="ExternalInput").ap()
    wk = nc.dram_tensor("wk", [QD, D2], BF16, kind="ExternalInput").ap()
    wv = nc.dram_tensor("wv", [QD, D2], BF16, kind="ExternalInput").ap()
    wo = nc.dram_tensor("wo", [D2, QD], BF16, kind="ExternalInput").ap()
    bsel_d = nc.dram_tensor("bsel", [8, 4 * 128], BF16, kind="ExternalInput").ap()
    out = nc.dram_tensor("out", [N, QD], F32, kind="ExternalOutput").ap()

    with TileContext(nc) as tc:
        with tc.tile_pool(name="persist", bufs=1) as pp, \
             tc.tile_pool(name="st_ps", bufs=2, space="PSUM") as st_ps, \
             tc.tile_pool(name="av_ps", bufs=1, space="PSUM") as av_ps, \
             tc.tile_pool(name="aux_ps", bufs=1, space="PSUM") as aux_ps, \
             tc.tile_pool(name="p0_sb", bufs=8) as p0_sb, \
             tc.tile_pool(name="n_sb", bufs=2) as n_sb:
            x_sb = pp.tile([128, KT, N], BF16)
            wq_sb = pp.tile([128, KT, D2], BF16)
            wk_sb = pp.tile([128, KT, D2], BF16)
            wv_sb = pp.tile([128, KT, D2], BF16)
            wo_sb = pp.tile([128, QD], BF16)
            ident = pp.tile([128, 128], BF16)
            identF = pp.tile([128, 128], F32)
            qT = pp.tile([128, N], BF16)
            kT = pp.tile([128, N], BF16)
            vT = pp.tile([128, N], BF16)
            v0p = pp.tile([128, NJ, DIM_HEAD + 1], BF16)
            v1p = pp.tile([128, NJ, DIM_HEAD + 1], BF16)
            ones_col = pp.tile([128, 1], F32)
            ones_bf = pp.tile([128, 1], BF16)
            bsel = pp.tile([8, 4 * 128], BF16)
            nc.sync.dma_start(out=bsel[:], in_=bsel_d[:])

            xTr = xT.rearrange("(k p) n -> p k n", p=128)
            # x slice 0 + wk + wq gate the first QK^T: issue them first.
            nc.sync.dma_start(out=x_sb[:, :, 0:ISL], in_=xTr[:, :, 0:ISL])
            nc.sync.dma_start(out=wk_sb[:], in_=wk.rearrange("(k p) m -> p k m", p=128))
            nc.sync.dma_start(out=wq_sb[:], in_=wq.rearrange("(k p) m -> p k m", p=128))
            nc.sync.dma_start(out=wv_sb[:], in_=wv.rearrange("(k p) m -> p k m", p=128))
            for s in range(1, NI):
                ssl = slice(s * ISL, (s + 1) * ISL)
                nc.sync.dma_start(out=x_sb[:, :, ssl], in_=xTr[:, :, ssl])
            nc.sync.dma_start(out=wo_sb[:], in_=wo[:])
            make_identity(nc, ident[:])
            make_identity(nc, identF[:])
            nc.gpsimd.memset(v0p[:, :, DIM_HEAD], 1.0)
            nc.gpsimd.memset(v1p[:, :, DIM_HEAD], 1.0)
            nc.gpsimd.memset(ones_col[:], 1.0)
            nc.gpsimd.memset(ones_bf[:], 1.0)

            def proj(w_sb, dst, s, tag="ps"):
                """dst[:, s*ISL:(s+1)*ISL] = (W^T @ x^T) slice via psum."""
                ssl = slice(s * ISL, (s + 1) * ISL)
                ps = aux_ps.tile([128, ISL], F32, tag=tag, name="projps") if tag != "st" \
                    else st_ps.tile([128, ISL], F32, tag="st", name="projst")
                for k in range(KT):
                    nc.tensor.matmul(ps[:], w_sb[:, k, :], x_sb[:, k, ssl],
                                     start=(k == 0), stop=(k == KT - 1))
                nc.vector.tensor_copy(out=dst[:, ssl], in_=ps[:])

            def transp(j):
                """V'[j] tiles from vT via PE transpose (both heads)."""
                tp = aux_ps.tile([128, 128], BF16, tag="aux", name="tp")
                nc.tensor.transpose(tp[:], vT[:, j * JTL:(j + 1) * JTL], ident[:])
                nc.vector.tensor_copy(out=v0p[:, j, 0:DIM_HEAD], in_=tp[:, 0:DIM_HEAD])
                nc.vector.tensor_copy(out=v1p[:, j, 0:DIM_HEAD], in_=tp[:, DIM_HEAD:D2])

            states = {}

            def epi(i_prev, step, tail=False):
                """Deferred epilogue for slice i_prev (runs during i_prev+1)."""
                e = states[i_prev]
                if step == 0:  # av -> SBUF (frees av PSUM; source for den/lh)
                    e["av_sb0"] = n_sb.tile([DIM_HEAD + 1, ISL], F32, tag="av_sb0", name="av_sb0")
                    e["av_sb1"] = n_sb.tile([DIM_HEAD + 1, ISL], F32, tag="av_sb1", name="av_sb1")
                    nc.vector.tensor_copy(out=e["av_sb0"][:], in_=e["av0"][:])
                    nc.vector.tensor_copy(out=e["av_sb1"][:], in_=e["av1"][:])
                elif step == 1:  # den rows to bf16 (one partition each)
                    e["db0"] = n_sb.tile([1, ISL], BF16, tag="db0", name="db0")
                    e["db1"] = n_sb.tile([1, ISL], BF16, tag="db1", name="db1")
                    nc.vector.tensor_copy(out=e["db0"][:],
                                          in_=e["av_sb0"][DIM_HEAD:DIM_HEAD + 1, :])
                    nc.vector.tensor_copy(out=e["db1"][:],
                                          in_=e["av_sb1"][DIM_HEAD:DIM_HEAD + 1, :])
                elif step == 11:  # den rows -> partitions via eight K=1 matmuls
                    e["dnt"] = aux_ps.tile([128, 8], F32, tag="ps", name="dnt")
                    for s in range(4):
                        for h in range(2):
                            db = e["db0"] if h == 0 else e["db1"]
                            c = 2 * s + h
                            nc.tensor.matmul(
                                e["dnt"][:, c:c + 1],
                                db[0:1, s * 128:(s + 1) * 128],
                                ones_bf[0:1, 0:1],
                                start=True, stop=True)
                elif step == 2:  # one wide reciprocal (0.2us vs 2x 3.3us)
                    e["rT"] = n_sb.tile([128, 8], F32, tag="rT", name="rT")
                    nc.vector.reciprocal(e["rT"][:], e["dnt"][:])
                elif step == 3:  # transpose rT back: [8, 128] = rT^T
                    e["rtt_ps"] = aux_ps.tile([8, 128], F32, tag="ps", name="rtt_ps")
                    nc.tensor.matmul(e["rtt_ps"][:], e["rT"][:], identF[:],
                                     start=True, stop=True)
                elif step == 4:
                    e["rtt"] = n_sb.tile([8, 128], BF16, tag="rtt", name="rtt")
                    nc.vector.tensor_copy(out=e["rtt"][:], in_=e["rtt_ps"][:])
                elif step == 5:  # broadcast recips along d2 via 4 PE matmuls
                    e["rb"] = aux_ps.tile([128, ISL], F32, tag="aux", name="rb")
                    for s in range(4):
                        nc.tensor.matmul(e["rb"][:, s * 128:(s + 1) * 128],
                                         bsel[:, s * 128:(s + 1) * 128],
                                         e["rtt"][:], start=True, stop=True)
                elif step == 6:  # normalize -> lh (bf16)
                    e["lh"] = n_sb.tile([128, ISL], BF16, tag="lh", name="lh")
                    nc.vector.tensor_mul(out=e["lh"][0:64, :],
                                         in0=e["av_sb0"][0:DIM_HEAD, :],
                                         in1=e["rb"][0:64, :])
                    nc.vector.tensor_mul(out=e["lh"][64:128, :],
                                         in0=e["av_sb1"][0:DIM_HEAD, :],
                                         in1=e["rb"][64:128, :])
                else:  # steps 7..10: one Wo matmul + store each
                    s = step - 7
                    if tail:  # st pool is idle at the tail: wider ladder
                        wop = st_ps.tile([128, QD], F32, tag="st", name=f"wot{s}")
                    else:
                        wop = aux_ps.tile([128, QD], F32, tag="aux", name="wop")
                    nc.tensor.matmul(wop[:], e["lh"][:, s * 128:(s + 1) * 128],
                                     wo_sb[:], start=True, stop=True)
                    wos = n_sb.tile([128, QD], F32, tag="wos", name="wos", bufs=4)
                    nc.vector.tensor_copy(out=wos[:], in_=wop[:])
                    nc.sync.dma_start(
                        out=out[i_prev * ISL + s * 128:i_prev * ISL + (s + 1) * 128, :],
                        in_=wos[:])

            # epilogue emission slots (j within the following slice)
            EPI = {6: 0, 7: 1, 8: 11, 9: 2, 12: 3, 13: 4, 14: 5, 15: 6,
                   16: 7, 18: 8, 20: 9, 22: 10}

            # virtual-clock pacing (see module docstring)
            HEAD_US = 13.0
            PACE0_US = 1.65
            PACE_US = 1.16

            def slot_ms(g):
                if g < NJ:
                    return (HEAD_US + g * PACE0_US) / 1e3
                return (HEAD_US + NJ * PACE0_US + (g - NJ) * PACE_US) / 1e3

            # warm-up: k and q projections for slice 0 (independent st slots)
            proj(wk_sb, kT, 0, tag="st")
            proj(wq_sb, qT, 0, tag="st")

            octs = {}  # (i, o) -> dict with sga/sgb/pta/ptb/ptu tiles

            def p_src(ga):
                """(ap, col0) holding exp'd tile ga for the AV matmuls."""
                return octs[ga], 0

            for g in range(TOT + LAG):
                tc.tile_set_cur_wait(ms=slot_ms(g))
                i, j = divmod(g, NJ) if g < TOT else (NI, g - TOT)
                if g < TOT:
                    if j == 0:
                        states[i] = {
                            "av0": av_ps.tile([DIM_HEAD + 1, ISL], F32, tag="av0", name="av0"),
                            "av1": av_ps.tile([DIM_HEAD + 1, ISL], F32, tag="av1", name="av1"),
                        }
                    isl = slice(i * ISL, (i + 1) * ISL)
                    jsl = slice(j * JTL, (j + 1) * JTL)
                    st = st_ps.tile([128, 2 * ISL], F32, tag="st", name="st")
                    nc.tensor.matmul(st[:, 0:ISL], kT[0:64, jsl], qT[0:64, isl],
                                     start=True, stop=True)
                    nc.tensor.matmul(st[:, ISL:2 * ISL], kT[64:128, jsl],
                                     qT[64:128, isl], start=True, stop=True)
                    pt = p0_sb.tile([128, 2 * ISL], BF16, tag="pt0", name="pt0")
                    nc.scalar.activation(pt[:], st[:], EXP, scale=SCALE)
                    octs[g] = pt
                if g >= LAG:
                    ga = g - LAG
                    ia, ja = divmod(ga, NJ)
                    src, c0 = p_src(ga)
                    eia = states[ia]
                    nc.tensor.matmul(eia["av0"][:], v0p[:, ja, :],
                                     src[:, c0:c0 + ISL],
                                     start=(ja == 0), stop=(ja == NJ - 1))
                    nc.tensor.matmul(eia["av1"][:], v1p[:, ja, :],
                                     src[:, c0 + ISL:c0 + 2 * ISL],
                                     start=(ja == 0), stop=(ja == NJ - 1))
                # slice-0 prologue: stream k/v/V' production
                if i == 0:
                    if j == 0:
                        proj(wv_sb, vT, 0)
                    elif j == 1:
                        proj(wk_sb, kT, 1)
                    elif j == 2:
                        proj(wv_sb, vT, 1)
                    elif j == 3:
                        for jj in range(4):
                            transp(jj)
                    elif j == 4:
                        for jj in range(4, 8):
                            transp(jj)
                    elif j >= 5 and j % 4 in (1, 2, 3):
                        s = j // 4 + 1
                        if s < NI:
                            if j % 4 == 1:
                                proj(wk_sb, kT, s)
                            elif j % 4 == 2:
                                proj(wv_sb, vT, s)
                            elif j > 5:
                                for jj in range(4 * s, 4 * s + 4):
                                    transp(jj)
                # deferred epilogue of slice i-1
                if 1 <= i < NI and j in EPI:
                    epi(i - 1, EPI[j])
                # next slice's q projection
                if g < TOT and j == 10 and i + 1 < NI:
                    proj(wq_sb, qT, i + 1)

            # tail: full epilogue for the last slice
            for sidx, step in enumerate((0, 1, 11, 2, 3, 4, 5, 6, 7, 8, 9, 10)):
                tc.tile_set_cur_wait(ms=slot_ms(TOT + LAG) + 0.0002 * sidx)
                epi(NI - 1, step, tail=True)

    nc.compile()
    return nc


_NC = None


def _get_program():
    global _NC
    if _NC is None:
        _NC = build_program()
    return _NC


def _bsel_host():
    b = np.zeros((8, 512), dtype=np.float32)
    for s in range(4):
        b[2 * s, s * 128:s * 128 + 64] = 1.0
        b[2 * s + 1, s * 128 + 64:(s + 1) * 128] = 1.0
    return b.astype(BFNP)


def kernel(x, Wq, Wk, Wv, Wo, bo):
    x = np.asarray(x, dtype=np.float32)
    Wq = np.asarray(Wq, dtype=np.float32)
    Wk = np.asarray(Wk, dtype=np.float32)
    Wv = np.asarray(Wv, dtype=np.float32)
    Wo = np.asarray(Wo, dtype=np.float32)
    bo = np.asarray(bo, dtype=np.float32)

    nc = _get_program()

    in_maps = []
    for c in range(NCORES):
        b, m = divmod(c, NCORES // B)
        cs = slice(m * D2, (m + 1) * D2)
        in_maps.append({
            "xT": np.ascontiguousarray(x[b].T).astype(BFNP),
            "wq": np.ascontiguousarray(Wq[:, cs]).astype(BFNP),
            "wk": np.ascontiguousarray(Wk[:, cs]).astype(BFNP),
            "wv": np.ascontiguousarray(Wv[:, cs]).astype(BFNP),
            "wo": np.ascontiguousarray(Wo[cs, :]).astype(BFNP),
            "bsel": _bsel_host(),
        })

    res = run_bass_kernel_spmd(nc, in_maps, core_ids=list(range(NCORES)))

    out = np.zeros((B, N, QD), dtype=np.float32)
    for c in range(NCORES):
        b = c // (NCORES // B)
        out[b] += res.results[c]["out"]
    out += bo[None, None, :]
    return out


# revision 21
# speedup vs baseline: 1.0601x; 1.0189x over previous
"""Trainium2 Bass kernel for nn_CrossAttention (self-attention, B=2, N=4096,
QD=512, 8 heads x 64 dim).

Sharding: 16 (batch, head) pairs across 8 cores -> core c handles batch c//4
and heads {2*(c%4), 2*(c%4)+1}.  Projection weights are column-sliced (Wq/Wk/Wv)
and row-sliced (Wo) per core; each core emits a partial [4096, 512] output that
the host sums per batch (row-parallel Wo => all-reduce done on host at gather).

ScalarE exp is the bottleneck engine (256 ACTIVATEs over [128,1024], ~1.1us
each, ~266us busy); everything else is scheduled to keep it fed:

  - Flat software pipeline over all 256 j-tiles: the next slice's QK^T is
    emitted before this slice's AV drain, so slice boundaries do not stall.
  - Per j-tile: row-tiled QK^T pair (K=64 heads at PE row groups 0/64) ->
    S^T [128j, 1024] fp32 in a 2-bank PSUM group; one exp ACTIVATE (scale
    fused, no max subtraction, |S| <~ 2); AV matmuls (M=65 with a ones
    column in V\' so softmax denominators fall out as row 64) lag by LAG=4.
  - Epilogue per slice is built from small ops only, so the strict-FIFO
    engine queues never head-block: den rows -> bf16 -> eight K=1 PE matmuls
    put them on partitions -> one [128,8] DVE reciprocal (0.2us; replaces
    two 3.3us single-partition iterative divides) -> PE transpose -> four PE
    broadcast matmuls -> normalize muls -> Wo -> DMA out.
  - Tile-scheduler virtual clock (tile_set_cur_wait) pins every iteration to
    its real-time slot; without it the compile-time scheduler hoists future
    slices\' work into earlier engine-queue positions (its cost model drifts
    from HW pace) and the DVE FIFO head-blocks for ~7us at every slice
    boundary, which also HAM-rethrottles the PE to 1.2GHz.
  - DMA order: x slice 0 + wk + wq first (they gate the first projection);
    all on the sync queue so x slice 0 gets full DMA bandwidth.

Measured: 318.7us HW exec (baseline 385.1us), rel err 4.7e-3.
"""

import sys

sys.path.insert(0, "/opt/trn_rl_repo")

import numpy as np
import ml_dtypes

import concourse.bass as bass
import concourse.mybir as mybir
from concourse import bacc
from concourse.tile import TileContext
from concourse.bass_utils import run_bass_kernel_spmd
from concourse.masks import make_identity

B, N, QD = 2, 4096, 512
HEADS, DIM_HEAD = 8, 64
INNER = HEADS * DIM_HEAD
SCALE = DIM_HEAD**-0.5

NCORES = 8
HPC = 2  # heads per core
D2 = HPC * DIM_HEAD  # 128
KT = 4  # k tiles of 128 over QD=512
ISL = 512  # i slice
NI = N // ISL  # 8
JTL = 128  # j tile
NJ = N // JTL  # 32
LAG = 4  # AV matmuls trail QK/exp by this many j-tiles
TOT = NI * NJ  # 256

F32 = mybir.dt.float32
BF16 = mybir.dt.bfloat16
BFNP = ml_dtypes.bfloat16
EXP = mybir.ActivationFunctionType.Exp

# exp staging pattern within each octet of j-tiles (slices >= 1): pos 0-3
# copied PSUM->SBUF by the DVE, exp'd in one [128,4096] ACTIVATE; pos 4-7
# exp'd straight from PSUM.  (GPSIMD cannot access PSUM, so the DVE is the
# only spare stager; 4/8 staged keeps it ~7us/slice under the ACT pace.)


def build_program():
    nc = bacc.Bacc("TRN2", target_bir_lowering=False, debug=False,
                   num_devices=NCORES)

    xT = nc.dram_tensor("xT", [QD, N], BF16, kind="ExternalInput").ap()
    wq = nc.dram_tensor("wq", [QD, D2], BF16, kind="ExternalInput").ap()
    wk = nc.dram_tensor("wk", [QD, D2], BF16, kind="ExternalInput").ap()
    wv = nc.dram_tensor("wv", [QD, D2], BF16, kind="ExternalInput").ap()
    wo = nc.dram_tensor("wo", [D2, QD], BF16, kind="ExternalInput").ap()
    bsel_d = nc.dram_tensor("bsel", [8, 4 * 128], BF16, kind="ExternalInput").ap()
    out = nc.dram_tensor("out", [N, QD], F32, kind="ExternalOutput").ap()

    with TileContext(nc) as tc:
        with tc.tile_pool(name="persist", bufs=1) as pp, \
             tc.tile_pool(name="st_ps", bufs=2, space="PSUM") as st_ps, \
             tc.tile_pool(name="av_ps", bufs=1, space="PSUM") as av_ps, \
             tc.tile_pool(name="aux_ps", bufs=1, space="PSUM") as aux_ps, \
             tc.tile_pool(name="p0_sb", bufs=8) as p0_sb, \
             tc.tile_pool(name="n_sb", bufs=2) as n_sb:
            x_sb = pp.tile([128, KT, N], BF16)
            wq_sb = pp.tile([128, KT, D2], BF16)
            wk_sb = pp.tile([128, KT, D2], BF16)
            wv_sb = pp.tile([128, KT, D2], BF16)
            wo_sb = pp.tile([128, QD], BF16)
            ident = pp.tile([128, 128], BF16)
            identF = pp.tile([128, 128], F32)
            qT = pp.tile([128, N], BF16)
            kT = pp.tile([128, N], BF16)
            vT = pp.tile([128, N], BF16)
            v0p = pp.tile([128, NJ, DIM_HEAD + 1], BF16)
            v1p = pp.tile([128, NJ, DIM_HEAD + 1], BF16)
            ones_col = pp.tile([128, 1], F32)
            ones_bf = pp.tile([128, 1], BF16)
            bsel = pp.tile([8, 4 * 128], BF16)
            nc.sync.dma_start(out=bsel[:], in_=bsel_d[:])

            xTr = xT.rearrange("(k p) n -> p k n", p=128)
            # x slice 0 + wk + wq gate the first QK^T: issue them first, with
            # x slice 0 split into its four k-chunks so the k-projection's
            # accumulation matmuls start as each chunk lands.
            nc.sync.dma_start(out=wk_sb[:], in_=wk.rearrange("(k p) m -> p k m", p=128))
            nc.sync.dma_start(out=x_sb[:, 0, 0:ISL], in_=xTr[:, 0, 0:ISL])
            nc.sync.dma_start(out=x_sb[:, 1, 0:ISL], in_=xTr[:, 1, 0:ISL])
            nc.sync.dma_start(out=wq_sb[:], in_=wq.rearrange("(k p) m -> p k m", p=128))
            nc.sync.dma_start(out=x_sb[:, 2, 0:ISL], in_=xTr[:, 2, 0:ISL])
            nc.sync.dma_start(out=x_sb[:, 3, 0:ISL], in_=xTr[:, 3, 0:ISL])
            nc.sync.dma_start(out=wv_sb[:], in_=wv.rearrange("(k p) m -> p k m", p=128))
            for s in range(1, NI):
                ssl = slice(s * ISL, (s + 1) * ISL)
                nc.sync.dma_start(out=x_sb[:, :, ssl], in_=xTr[:, :, ssl])
            nc.sync.dma_start(out=wo_sb[:], in_=wo[:])
            make_identity(nc, ident[:])
            make_identity(nc, identF[:])
            nc.gpsimd.memset(v0p[:, :, DIM_HEAD], 1.0)
            nc.gpsimd.memset(v1p[:, :, DIM_HEAD], 1.0)
            nc.gpsimd.memset(ones_col[:], 1.0)
            nc.gpsimd.memset(ones_bf[:], 1.0)

            def proj(w_sb, dst, s, tag="ps"):
                """dst[:, s*ISL:(s+1)*ISL] = (W^T @ x^T) slice via psum."""
                ssl = slice(s * ISL, (s + 1) * ISL)
                ps = aux_ps.tile([128, ISL], F32, tag=tag, name="projps") if tag != "st" \
                    else st_ps.tile([128, ISL], F32, tag="st", name="projst")
                for k in range(KT):
                    nc.tensor.matmul(ps[:], w_sb[:, k, :], x_sb[:, k, ssl],
                                     start=(k == 0), stop=(k == KT - 1))
                nc.vector.tensor_copy(out=dst[:, ssl], in_=ps[:])

            def transp(j):
                """V'[j] tiles from vT via PE transpose (both heads)."""
                tp = aux_ps.tile([128, 128], BF16, tag="aux", name="tp")
                nc.tensor.transpose(tp[:], vT[:, j * JTL:(j + 1) * JTL], ident[:])
                nc.vector.tensor_copy(out=v0p[:, j, 0:DIM_HEAD], in_=tp[:, 0:DIM_HEAD])
                nc.vector.tensor_copy(out=v1p[:, j, 0:DIM_HEAD], in_=tp[:, DIM_HEAD:D2])

            states = {}

            def epi(i_prev, step, tail=False):
                """Deferred epilogue for slice i_prev (runs during i_prev+1)."""
                e = states[i_prev]
                if step == 0:  # av -> SBUF (frees av PSUM; source for den/lh)
                    e["av_sb0"] = n_sb.tile([DIM_HEAD + 1, ISL], F32, tag="av_sb0", name="av_sb0")
                    e["av_sb1"] = n_sb.tile([DIM_HEAD + 1, ISL], F32, tag="av_sb1", name="av_sb1")
                    nc.vector.tensor_copy(out=e["av_sb0"][:], in_=e["av0"][:])
                    nc.vector.tensor_copy(out=e["av_sb1"][:], in_=e["av1"][:])
                elif step == 1:  # den rows to bf16 (one partition each)
                    e["db0"] = n_sb.tile([1, ISL], BF16, tag="db0", name="db0")
                    e["db1"] = n_sb.tile([1, ISL], BF16, tag="db1", name="db1")
                    nc.vector.tensor_copy(out=e["db0"][:],
                                          in_=e["av_sb0"][DIM_HEAD:DIM_HEAD + 1, :])
                    nc.vector.tensor_copy(out=e["db1"][:],
                                          in_=e["av_sb1"][DIM_HEAD:DIM_HEAD + 1, :])
                elif step == 11:  # den rows -> partitions via eight K=1 matmuls
                    e["dnt"] = aux_ps.tile([128, 8], F32, tag="ps", name="dnt")
                    for s in range(4):
                        for h in range(2):
                            db = e["db0"] if h == 0 else e["db1"]
                            c = 2 * s + h
                            nc.tensor.matmul(
                                e["dnt"][:, c:c + 1],
                                db[0:1, s * 128:(s + 1) * 128],
                                ones_bf[0:1, 0:1],
                                start=True, stop=True)
                elif step == 2:  # one wide reciprocal (0.2us vs 2x 3.3us)
                    e["rT"] = n_sb.tile([128, 8], F32, tag="rT", name="rT")
                    nc.vector.reciprocal(e["rT"][:], e["dnt"][:])
                elif step == 3:  # transpose rT back: [8, 128] = rT^T
                    e["rtt_ps"] = aux_ps.tile([8, 128], F32, tag="ps", name="rtt_ps")
                    nc.tensor.matmul(e["rtt_ps"][:], e["rT"][:], identF[:],
                                     start=True, stop=True)
                elif step == 4:
                    e["rtt"] = n_sb.tile([8, 128], BF16, tag="rtt", name="rtt")
                    nc.vector.tensor_copy(out=e["rtt"][:], in_=e["rtt_ps"][:])
                elif step == 5:  # broadcast recips along d2 via 4 PE matmuls
                    e["rb"] = aux_ps.tile([128, ISL], F32, tag="aux", name="rb")
                    for s in range(4):
                        nc.tensor.matmul(e["rb"][:, s * 128:(s + 1) * 128],
                                         bsel[:, s * 128:(s + 1) * 128],
                                         e["rtt"][:], start=True, stop=True)
                elif step == 6:  # normalize -> lh (bf16)
                    e["lh"] = n_sb.tile([128, ISL], BF16, tag="lh", name="lh")
                    nc.vector.tensor_mul(out=e["lh"][0:64, :],
                                         in0=e["av_sb0"][0:DIM_HEAD, :],
                                         in1=e["rb"][0:64, :])
                    nc.vector.tensor_mul(out=e["lh"][64:128, :],
                                         in0=e["av_sb1"][0:DIM_HEAD, :],
                                         in1=e["rb"][64:128, :])
                else:  # steps 7..10: one Wo matmul + store each
                    s = step - 7
                    if tail:  # st pool is idle at the tail: wider ladder
                        wop = st_ps.tile([128, QD], F32, tag="st", name=f"wot{s}")
                    else:
                        wop = aux_ps.tile([128, QD], F32, tag="aux", name="wop")
                    nc.tensor.matmul(wop[:], e["lh"][:, s * 128:(s + 1) * 128],
                                     wo_sb[:], start=True, stop=True)
                    wos = n_sb.tile([128, QD], F32, tag="wos", name="wos", bufs=4)
                    nc.vector.tensor_copy(out=wos[:], in_=wop[:])
                    nc.sync.dma_start(
                        out=out[i_prev * ISL + s * 128:i_prev * ISL + (s + 1) * 128, :],
                        in_=wos[:])

            # epilogue emission slots (j within the following slice)
            EPI = {4: 0, 7: 1, 8: 11, 9: 2, 12: 3, 13: 4, 14: 5, 15: 6,
                   16: 7, 18: 8, 20: 9, 22: 10}

            # virtual-clock pacing (see module docstring)
            HEAD_US = 13.0
            PACE0_US = 1.65
            PACE_US = 1.16

            def slot_ms(g):
                if g < NJ:
                    return (HEAD_US + g * PACE0_US) / 1e3
                return (HEAD_US + NJ * PACE0_US + (g - NJ) * PACE_US) / 1e3

            # warm-up: k and q projections for slice 0 (independent st slots)
            proj(wk_sb, kT, 0, tag="st")
            proj(wq_sb, qT, 0, tag="st")

            octs = {}  # (i, o) -> dict with sga/sgb/pta/ptb/ptu tiles

            def p_src(ga):
                """(ap, col0) holding exp'd tile ga for the AV matmuls."""
                return octs[ga], 0

            for g in range(TOT + LAG):
                tc.tile_set_cur_wait(ms=slot_ms(g))
                i, j = divmod(g, NJ) if g < TOT else (NI, g - TOT)
                if g < TOT:
                    if j == 0:
                        states[i] = {
                            "av0": av_ps.tile([DIM_HEAD + 1, ISL], F32, tag="av0", name="av0"),
                            "av1": av_ps.tile([DIM_HEAD + 1, ISL], F32, tag="av1", name="av1"),
                        }
                    isl = slice(i * ISL, (i + 1) * ISL)
                    jsl = slice(j * JTL, (j + 1) * JTL)
                    st = st_ps.tile([128, 2 * ISL], F32, tag="st", name="st")
                    nc.tensor.matmul(st[:, 0:ISL], kT[0:64, jsl], qT[0:64, isl],
                                     start=True, stop=True)
                    nc.tensor.matmul(st[:, ISL:2 * ISL], kT[64:128, jsl],
                                     qT[64:128, isl], start=True, stop=True)
                    pt = p0_sb.tile([128, 2 * ISL], BF16, tag="pt0", name="pt0")
                    nc.scalar.activation(pt[:], st[:], EXP, scale=SCALE)
                    octs[g] = pt
                if g >= LAG:
                    ga = g - LAG
                    ia, ja = divmod(ga, NJ)
                    src, c0 = p_src(ga)
                    eia = states[ia]
                    nc.tensor.matmul(eia["av0"][:], v0p[:, ja, :],
                                     src[:, c0:c0 + ISL],
                                     start=(ja == 0), stop=(ja == NJ - 1))
                    nc.tensor.matmul(eia["av1"][:], v1p[:, ja, :],
                                     src[:, c0 + ISL:c0 + 2 * ISL],
                                     start=(ja == 0), stop=(ja == NJ - 1))
                # slice-0 prologue: stream k/v/V' production
                if i == 0:
                    if j == 0:
                        proj(wv_sb, vT, 0)
                    elif j == 1:
                        proj(wk_sb, kT, 1)
                    elif j == 2:
                        proj(wv_sb, vT, 1)
                    elif j == 3:
                        for jj in range(4):
                            transp(jj)
                    elif j == 4:
                        for jj in range(4, 8):
                            transp(jj)
                    elif j >= 5 and j % 4 in (1, 2, 3):
                        s = j // 4 + 1
                        if s < NI:
                            if j % 4 == 1:
                                proj(wk_sb, kT, s)
                            elif j % 4 == 2:
                                proj(wv_sb, vT, s)
                            elif j > 5:
                                for jj in range(4 * s, 4 * s + 4):
                                    transp(jj)
                # deferred epilogue of slice i-1
                if 1 <= i < NI and j in EPI:
                    epi(i - 1, EPI[j])
                # next slice's q projection
                if g < TOT and j == 10 and i + 1 < NI:
                    proj(wq_sb, qT, i + 1)

            # tail: full epilogue for the last slice
            for sidx, step in enumerate((0, 1, 11, 2, 3, 4, 5, 6, 7, 8, 9, 10)):
                tc.tile_set_cur_wait(ms=slot_ms(TOT + LAG) + 0.0002 * sidx)
                epi(NI - 1, step, tail=True)

    nc.compile()
    return nc


_NC = None


def _get_program():
    global _NC
    if _NC is None:
        _NC = build_program()
    return _NC


def _bsel_host():
    b = np.zeros((8, 512), dtype=np.float32)
    for s in range(4):
        b[2 * s, s * 128:s * 128 + 64] = 1.0
        b[2 * s + 1, s * 128 + 64:(s + 1) * 128] = 1.0
    return b.astype(BFNP)


def kernel(x, Wq, Wk, Wv, Wo, bo):
    x = np.asarray(x, dtype=np.float32)
    Wq = np.asarray(Wq, dtype=np.float32)
    Wk = np.asarray(Wk, dtype=np.float32)
    Wv = np.asarray(Wv, dtype=np.float32)
    Wo = np.asarray(Wo, dtype=np.float32)
    bo = np.asarray(bo, dtype=np.float32)

    nc = _get_program()

    in_maps = []
    for c in range(NCORES):
        b, m = divmod(c, NCORES // B)
        cs = slice(m * D2, (m + 1) * D2)
        in_maps.append({
            "xT": np.ascontiguousarray(x[b].T).astype(BFNP),
            "wq": np.ascontiguousarray(Wq[:, cs]).astype(BFNP),
            "wk": np.ascontiguousarray(Wk[:, cs]).astype(BFNP),
            "wv": np.ascontiguousarray(Wv[:, cs]).astype(BFNP),
            "wo": np.ascontiguousarray(Wo[cs, :]).astype(BFNP),
            "bsel": _bsel_host(),
        })

    res = run_bass_kernel_spmd(nc, in_maps, core_ids=list(range(NCORES)))

    out = np.zeros((B, N, QD), dtype=np.float32)
    for c in range(NCORES):
        b = c // (NCORES // B)
        out[b] += res.results[c]["out"]
    out += bo[None, None, :]
    return out


# revision 23
# speedup vs baseline: 1.0607x; 1.0006x over previous
"""Trainium2 Bass kernel for nn_CrossAttention (self-attention, B=2, N=4096,
QD=512, 8 heads x 64 dim).

Sharding: 16 (batch, head) pairs across 8 cores -> core c handles batch c//4
and heads {2*(c%4), 2*(c%4)+1}.  Projection weights are column-sliced (Wq/Wk/Wv)
and row-sliced (Wo) per core; each core emits a partial [4096, 512] output that
the host sums per batch (row-parallel Wo => all-reduce done on host at gather).

ScalarE exp is the bottleneck engine (256 ACTIVATEs over [128,1024], ~1.1us
each, ~266us busy); everything else is scheduled to keep it fed:

  - Flat software pipeline over all 256 j-tiles: the next slice's QK^T is
    emitted before this slice's AV drain, so slice boundaries do not stall.
  - Per j-tile: row-tiled QK^T pair (K=64 heads at PE row groups 0/64) ->
    S^T [128j, 1024] fp32 in a 2-bank PSUM group; one exp ACTIVATE (scale
    fused, no max subtraction, |S| <~ 2); AV matmuls (M=65 with a ones
    column in V\' so softmax denominators fall out as row 64) lag by LAG=4.
  - Epilogue per slice is built from small ops only, so the strict-FIFO
    engine queues never head-block: den rows -> bf16 -> eight K=1 PE matmuls
    put them on partitions -> one [128,8] DVE reciprocal (0.2us; replaces
    two 3.3us single-partition iterative divides) -> PE transpose -> four PE
    broadcast matmuls -> normalize muls -> Wo -> DMA out.
  - Tile-scheduler virtual clock (tile_set_cur_wait) pins every iteration to
    its real-time slot; without it the compile-time scheduler hoists future
    slices\' work into earlier engine-queue positions (its cost model drifts
    from HW pace) and the DVE FIFO head-blocks for ~7us at every slice
    boundary, which also HAM-rethrottles the PE to 1.2GHz.
  - DMA order: wk + x slice 0 (split into k-chunks so the k-projection
    starts as each chunk lands) + wq first; all on one queue so the gating
    transfers get full DMA bandwidth (multi-queue issue was tried and lost:
    concurrent transfers starve the first-needed one).

Measured: 312.8us HW exec (baseline 385.1us), rel err 4.7e-3.
"""

import sys

sys.path.insert(0, "/opt/trn_rl_repo")

import numpy as np
import ml_dtypes

import concourse.bass as bass
import concourse.mybir as mybir
from concourse import bacc
from concourse.tile import TileContext
from concourse.bass_utils import run_bass_kernel_spmd
from concourse.masks import make_identity

B, N, QD = 2, 4096, 512
HEADS, DIM_HEAD = 8, 64
INNER = HEADS * DIM_HEAD
SCALE = DIM_HEAD**-0.5

NCORES = 8
HPC = 2  # heads per core
D2 = HPC * DIM_HEAD  # 128
KT = 4  # k tiles of 128 over QD=512
ISL = 512  # i slice
NI = N // ISL  # 8
JTL = 128  # j tile
NJ = N // JTL  # 32
LAG = 4  # AV matmuls trail QK/exp by this many j-tiles
TOT = NI * NJ  # 256

F32 = mybir.dt.float32
BF16 = mybir.dt.bfloat16
BFNP = ml_dtypes.bfloat16
EXP = mybir.ActivationFunctionType.Exp

# exp staging pattern within each octet of j-tiles (slices >= 1): pos 0-3
# copied PSUM->SBUF by the DVE, exp'd in one [128,4096] ACTIVATE; pos 4-7
# exp'd straight from PSUM.  (GPSIMD cannot access PSUM, so the DVE is the
# only spare stager; 4/8 staged keeps it ~7us/slice under the ACT pace.)


def build_program():
    nc = bacc.Bacc("TRN2", target_bir_lowering=False, debug=False,
                   num_devices=NCORES)

    xT = nc.dram_tensor("xT", [QD, N], BF16, kind="ExternalInput").ap()
    wq = nc.dram_tensor("wq", [QD, D2], BF16, kind="ExternalInput").ap()
    wk = nc.dram_tensor("wk", [QD, D2], BF16, kind="ExternalInput").ap()
    wv = nc.dram_tensor("wv", [QD, D2], BF16, kind="ExternalInput").ap()
    wo = nc.dram_tensor("wo", [D2, QD], BF16, kind="ExternalInput").ap()
    bsel_d = nc.dram_tensor("bsel", [8, 4 * 128], BF16, kind="ExternalInput").ap()
    out = nc.dram_tensor("out", [N, QD], F32, kind="ExternalOutput").ap()

    with TileContext(nc) as tc:
        with tc.tile_pool(name="persist", bufs=1) as pp, \
             tc.tile_pool(name="st_ps", bufs=2, space="PSUM") as st_ps, \
             tc.tile_pool(name="av_ps", bufs=1, space="PSUM") as av_ps, \
             tc.tile_pool(name="aux_ps", bufs=1, space="PSUM") as aux_ps, \
             tc.tile_pool(name="p0_sb", bufs=8) as p0_sb, \
             tc.tile_pool(name="n_sb", bufs=2) as n_sb:
            x_sb = pp.tile([128, KT, N], BF16)
            wq_sb = pp.tile([128, KT, D2], BF16)
            wk_sb = pp.tile([128, KT, D2], BF16)
            wv_sb = pp.tile([128, KT, D2], BF16)
            wo_sb = pp.tile([128, QD], BF16)
            ident = pp.tile([128, 128], BF16)
            identF = pp.tile([128, 128], F32)
            qT = pp.tile([128, N], BF16)
            kT = pp.tile([128, N], BF16)
            vT = pp.tile([128, N], BF16)
            v0p = pp.tile([128, NJ, DIM_HEAD + 1], BF16)
            v1p = pp.tile([128, NJ, DIM_HEAD + 1], BF16)
            ones_col = pp.tile([128, 1], F32)
            ones_bf = pp.tile([128, 1], BF16)
            bsel = pp.tile([8, 4 * 128], BF16)
            nc.sync.dma_start(out=bsel[:], in_=bsel_d[:])

            xTr = xT.rearrange("(k p) n -> p k n", p=128)
            # x slice 0 + wk + wq gate the first QK^T: issue them first, with
            # x slice 0 split into its four k-chunks so the k-projection's
            # accumulation matmuls start as each chunk lands.
            nc.sync.dma_start(out=wk_sb[:], in_=wk.rearrange("(k p) m -> p k m", p=128))
            nc.sync.dma_start(out=x_sb[:, 0, 0:ISL], in_=xTr[:, 0, 0:ISL])
            nc.sync.dma_start(out=x_sb[:, 1, 0:ISL], in_=xTr[:, 1, 0:ISL])
            nc.sync.dma_start(out=wq_sb[:], in_=wq.rearrange("(k p) m -> p k m", p=128))
            nc.sync.dma_start(out=x_sb[:, 2, 0:ISL], in_=xTr[:, 2, 0:ISL])
            nc.sync.dma_start(out=x_sb[:, 3, 0:ISL], in_=xTr[:, 3, 0:ISL])
            nc.sync.dma_start(out=wv_sb[:], in_=wv.rearrange("(k p) m -> p k m", p=128))
            for s in range(1, NI):
                ssl = slice(s * ISL, (s + 1) * ISL)
                nc.sync.dma_start(out=x_sb[:, :, ssl], in_=xTr[:, :, ssl])
            nc.sync.dma_start(out=wo_sb[:], in_=wo[:])
            make_identity(nc, ident[:])
            make_identity(nc, identF[:])
            nc.gpsimd.memset(v0p[:, :, DIM_HEAD], 1.0)
            nc.gpsimd.memset(v1p[:, :, DIM_HEAD], 1.0)
            nc.gpsimd.memset(ones_col[:], 1.0)
            nc.gpsimd.memset(ones_bf[:], 1.0)

            def proj(w_sb, dst, s, tag="ps"):
                """dst[:, s*ISL:(s+1)*ISL] = (W^T @ x^T) slice via psum."""
                ssl = slice(s * ISL, (s + 1) * ISL)
                ps = aux_ps.tile([128, ISL], F32, tag=tag, name="projps") if tag != "st" \
                    else st_ps.tile([128, ISL], F32, tag="st", name="projst")
                for k in range(KT):
                    nc.tensor.matmul(ps[:], w_sb[:, k, :], x_sb[:, k, ssl],
                                     start=(k == 0), stop=(k == KT - 1))
                nc.vector.tensor_copy(out=dst[:, ssl], in_=ps[:])

            def transp(j):
                """V'[j] tiles from vT via PE transpose (both heads)."""
                tp = aux_ps.tile([128, 128], BF16, tag="aux", name="tp")
                nc.tensor.transpose(tp[:], vT[:, j * JTL:(j + 1) * JTL], ident[:])
                nc.vector.tensor_copy(out=v0p[:, j, 0:DIM_HEAD], in_=tp[:, 0:DIM_HEAD])
                nc.vector.tensor_copy(out=v1p[:, j, 0:DIM_HEAD], in_=tp[:, DIM_HEAD:D2])

            states = {}

            def epi(i_prev, step, tail=False):
                """Deferred epilogue for slice i_prev (runs during i_prev+1)."""
                e = states[i_prev]
                if step == 0:  # av -> SBUF (frees av PSUM; source for den/lh)
                    e["av_sb0"] = n_sb.tile([DIM_HEAD + 1, ISL], F32, tag="av_sb0", name="av_sb0")
                    e["av_sb1"] = n_sb.tile([DIM_HEAD + 1, ISL], F32, tag="av_sb1", name="av_sb1")
                    nc.vector.tensor_copy(out=e["av_sb0"][:], in_=e["av0"][:])
                    nc.vector.tensor_copy(out=e["av_sb1"][:], in_=e["av1"][:])
                elif step == 1:  # den rows to bf16 (one partition each)
                    e["db0"] = n_sb.tile([1, ISL], BF16, tag="db0", name="db0")
                    e["db1"] = n_sb.tile([1, ISL], BF16, tag="db1", name="db1")
                    nc.vector.tensor_copy(out=e["db0"][:],
                                          in_=e["av_sb0"][DIM_HEAD:DIM_HEAD + 1, :])
                    nc.vector.tensor_copy(out=e["db1"][:],
                                          in_=e["av_sb1"][DIM_HEAD:DIM_HEAD + 1, :])
                elif step == 11:  # den rows -> partitions via eight K=1 matmuls
                    e["dnt"] = aux_ps.tile([128, 8], F32, tag="ps", name="dnt")
                    for s in range(4):
                        for h in range(2):
                            db = e["db0"] if h == 0 else e["db1"]
                            c = 2 * s + h
                            nc.tensor.matmul(
                                e["dnt"][:, c:c + 1],
                                db[0:1, s * 128:(s + 1) * 128],
                                ones_bf[0:1, 0:1],
                                start=True, stop=True)
                elif step == 2:  # one wide reciprocal (0.2us vs 2x 3.3us)
                    e["rT"] = n_sb.tile([128, 8], F32, tag="rT", name="rT")
                    nc.vector.reciprocal(e["rT"][:], e["dnt"][:])
                elif step == 3:  # transpose rT back: [8, 128] = rT^T
                    e["rtt_ps"] = aux_ps.tile([8, 128], F32, tag="ps", name="rtt_ps")
                    nc.tensor.matmul(e["rtt_ps"][:], e["rT"][:], identF[:],
                                     start=True, stop=True)
                elif step == 4:
                    e["rtt"] = n_sb.tile([8, 128], BF16, tag="rtt", name="rtt")
                    nc.vector.tensor_copy(out=e["rtt"][:], in_=e["rtt_ps"][:])
                elif step == 5:  # broadcast recips along d2 via 4 PE matmuls
                    e["rb"] = aux_ps.tile([128, ISL], F32, tag="aux", name="rb")
                    for s in range(4):
                        nc.tensor.matmul(e["rb"][:, s * 128:(s + 1) * 128],
                                         bsel[:, s * 128:(s + 1) * 128],
                                         e["rtt"][:], start=True, stop=True)
                elif step == 6:  # normalize -> lh (bf16), column-split so
                    # the first Wo matmuls only wait for lh cols 0..255
                    e["lh"] = n_sb.tile([128, ISL], BF16, tag="lh", name="lh")
                    for cs in (slice(0, 256), slice(256, 512)):
                        nc.vector.tensor_mul(out=e["lh"][0:64, cs],
                                             in0=e["av_sb0"][0:DIM_HEAD, cs],
                                             in1=e["rb"][0:64, cs])
                        nc.vector.tensor_mul(out=e["lh"][64:128, cs],
                                             in0=e["av_sb1"][0:DIM_HEAD, cs],
                                             in1=e["rb"][64:128, cs])
                else:  # steps 7..10: one Wo matmul + store each
                    s = step - 7
                    if tail:  # st pool is idle at the tail: wider ladder
                        wop = st_ps.tile([128, QD], F32, tag="st", name=f"wot{s}")
                    else:
                        wop = aux_ps.tile([128, QD], F32, tag="aux", name="wop")
                    nc.tensor.matmul(wop[:], e["lh"][:, s * 128:(s + 1) * 128],
                                     wo_sb[:], start=True, stop=True)
                    wos = n_sb.tile([128, QD], F32, tag="wos", name="wos", bufs=4)
                    nc.vector.tensor_copy(out=wos[:], in_=wop[:])
                    deng = nc.scalar if (tail and s % 2) else nc.sync
                    deng.dma_start(
                        out=out[i_prev * ISL + s * 128:i_prev * ISL + (s + 1) * 128, :],
                        in_=wos[:])

            # epilogue emission slots (j within the following slice)
            EPI = {4: 0, 7: 1, 8: 11, 9: 2, 12: 3, 13: 4, 14: 5, 15: 6,
                   16: 7, 18: 8, 20: 9, 22: 10}

            # virtual-clock pacing (see module docstring)
            HEAD_US = 13.0
            PACE0_US = 1.65
            PACE_US = 1.16

            def slot_ms(g):
                if g < NJ:
                    return (HEAD_US + g * PACE0_US) / 1e3
                return (HEAD_US + NJ * PACE0_US + (g - NJ) * PACE_US) / 1e3

            # warm-up: k and q projections for slice 0 (independent st slots)
            proj(wk_sb, kT, 0, tag="st")
            proj(wq_sb, qT, 0, tag="st")

            octs = {}  # (i, o) -> dict with sga/sgb/pta/ptb/ptu tiles

            def p_src(ga):
                """(ap, col0) holding exp'd tile ga for the AV matmuls."""
                return octs[ga], 0

            for g in range(TOT + LAG):
                tc.tile_set_cur_wait(ms=slot_ms(g))
                i, j = divmod(g, NJ) if g < TOT else (NI, g - TOT)
                if g < TOT:
                    if j == 0:
                        states[i] = {
                            "av0": av_ps.tile([DIM_HEAD + 1, ISL], F32, tag="av0", name="av0"),
                            "av1": av_ps.tile([DIM_HEAD + 1, ISL], F32, tag="av1", name="av1"),
                        }
                    isl = slice(i * ISL, (i + 1) * ISL)
                    jsl = slice(j * JTL, (j + 1) * JTL)
                    st = st_ps.tile([128, 2 * ISL], F32, tag="st", name="st")
                    nc.tensor.matmul(st[:, 0:ISL], kT[0:64, jsl], qT[0:64, isl],
                                     start=True, stop=True)
                    nc.tensor.matmul(st[:, ISL:2 * ISL], kT[64:128, jsl],
                                     qT[64:128, isl], start=True, stop=True)
                    pt = p0_sb.tile([128, 2 * ISL], BF16, tag="pt0", name="pt0")
                    nc.scalar.activation(pt[:], st[:], EXP, scale=SCALE)
                    octs[g] = pt
                if g >= LAG:
                    ga = g - LAG
                    ia, ja = divmod(ga, NJ)
                    src, c0 = p_src(ga)
                    eia = states[ia]
                    nc.tensor.matmul(eia["av0"][:], v0p[:, ja, :],
                                     src[:, c0:c0 + ISL],
                                     start=(ja == 0), stop=(ja == NJ - 1))
                    nc.tensor.matmul(eia["av1"][:], v1p[:, ja, :],
                                     src[:, c0 + ISL:c0 + 2 * ISL],
                                     start=(ja == 0), stop=(ja == NJ - 1))
                # slice-0 prologue: stream k/v/V' production
                if i == 0:
                    if j == 0:
                        proj(wv_sb, vT, 0)
                    elif j == 1:
                        proj(wk_sb, kT, 1)
                    elif j == 2:
                        proj(wv_sb, vT, 1)
                    elif j == 3:
                        for jj in range(4):
                            transp(jj)
                    elif j == 4:
                        for jj in range(4, 8):
                            transp(jj)
                    elif j >= 5 and j % 4 in (1, 2, 3):
                        s = j // 4 + 1
                        if s < NI:
                            if j % 4 == 1:
                                proj(wk_sb, kT, s)
                            elif j % 4 == 2:
                                proj(wv_sb, vT, s)
                            elif j > 5:
                                for jj in range(4 * s, 4 * s + 4):
                                    transp(jj)
                # deferred epilogue of slice i-1
                if 1 <= i < NI and j in EPI:
                    epi(i - 1, EPI[j])
                # next slice's q projection
                if g < TOT and j == 10 and i + 1 < NI:
                    proj(wq_sb, qT, i + 1)

            # tail: full epilogue for the last slice
            for sidx, step in enumerate((0, 1, 11, 2, 3, 4, 5, 6, 7, 8, 9, 10)):
                tc.tile_set_cur_wait(ms=slot_ms(TOT + LAG) + 0.0002 * sidx)
                epi(NI - 1, step, tail=True)

    nc.compile()
    return nc


_NC = None


def _get_program():
    global _NC
    if _NC is None:
        _NC = build_program()
    return _NC


def _bsel_host():
    b = np.zeros((8, 512), dtype=np.float32)
    for s in range(4):
        b[2 * s, s * 128:s * 128 + 64] = 1.0
        b[2 * s + 1, s * 128 + 64:(s + 1) * 128] = 1.0
    return b.astype(BFNP)


def kernel(x, Wq, Wk, Wv, Wo, bo):
    x = np.asarray(x, dtype=np.float32)
    Wq = np.asarray(Wq, dtype=np.float32)
    Wk = np.asarray(Wk, dtype=np.float32)
    Wv = np.asarray(Wv, dtype=np.float32)
    Wo = np.asarray(Wo, dtype=np.float32)
    bo = np.asarray(bo, dtype=np.float32)

    nc = _get_program()

    in_maps = []
    for c in range(NCORES):
        b, m = divmod(c, NCORES // B)
        cs = slice(m * D2, (m + 1) * D2)
        in_maps.append({
            "xT": np.ascontiguousarray(x[b].T).astype(BFNP),
            "wq": np.ascontiguousarray(Wq[:, cs]).astype(BFNP),
            "wk": np.ascontiguousarray(Wk[:, cs]).astype(BFNP),
            "wv": np.ascontiguousarray(Wv[:, cs]).astype(BFNP),
            "wo": np.ascontiguousarray(Wo[cs, :]).astype(BFNP),
            "bsel": _bsel_host(),
        })

    res = run_bass_kernel_spmd(nc, in_maps, core_ids=list(range(NCORES)))

    out = np.zeros((B, N, QD), dtype=np.float32)
    for c in range(NCORES):
        b = c // (NCORES // B)
        out[b] += res.results[c]["out"]
    out += bo[None, None, :]
    return out


# revision 26
# speedup vs baseline: 1.0649x; 1.0040x over previous
"""Trainium2 Bass kernel for nn_CrossAttention (self-attention, B=2, N=4096,
QD=512, 8 heads x 64 dim).

Sharding: 16 (batch, head) pairs across 8 cores -> core c handles batch c//4
and heads {2*(c%4), 2*(c%4)+1}.  Projection weights are column-sliced (Wq/Wk/Wv)
and row-sliced (Wo) per core; each core emits a partial [4096, 512] output that
the host sums per batch (row-parallel Wo => all-reduce done on host at gather).

ScalarE exp is the bottleneck engine (256 ACTIVATEs over [128,1024], ~1.1us
each, ~266us busy); everything else is scheduled to keep it fed:

  - Flat software pipeline over all 256 j-tiles: the next slice's QK^T is
    emitted before this slice's AV drain, so slice boundaries do not stall.
  - Per j-tile: row-tiled QK^T pair (K=64 heads at PE row groups 0/64) ->
    S^T [128j, 1024] fp32 in a 2-bank PSUM group; one exp ACTIVATE (scale
    fused, no max subtraction, |S| <~ 2); AV matmuls (M=65 with a ones
    column in V\' so softmax denominators fall out as row 64) lag by LAG=4.
  - Epilogue per slice is built from small ops only, so the strict-FIFO
    engine queues never head-block: den rows -> bf16 -> eight K=1 PE matmuls
    put them on partitions -> one [128,8] DVE reciprocal (0.2us; replaces
    two 3.3us single-partition iterative divides) -> PE transpose -> four PE
    broadcast matmuls -> normalize muls -> Wo -> DMA out.
  - Tile-scheduler virtual clock (tile_set_cur_wait) pins every iteration to
    its real-time slot; without it the compile-time scheduler hoists future
    slices\' work into earlier engine-queue positions (its cost model drifts
    from HW pace) and the DVE FIFO head-blocks for ~7us at every slice
    boundary, which also HAM-rethrottles the PE to 1.2GHz.
  - DMA order: wk + x slice 0 (split into k-chunks so the k-projection
    starts as each chunk lands) + wq first; all on one queue so the gating
    transfers get full DMA bandwidth (multi-queue issue was tried and lost:
    concurrent transfers starve the first-needed one).

Measured: 312.6us HW exec (baseline 385.1us), rel err 4.7e-3.
"""

import sys

sys.path.insert(0, "/opt/trn_rl_repo")

import numpy as np
import ml_dtypes

import concourse.bass as bass
import concourse.mybir as mybir
from concourse import bacc
from concourse.tile import TileContext
from concourse.bass_utils import run_bass_kernel_spmd
from concourse.masks import make_identity

B, N, QD = 2, 4096, 512
HEADS, DIM_HEAD = 8, 64
INNER = HEADS * DIM_HEAD
SCALE = DIM_HEAD**-0.5

NCORES = 8
HPC = 2  # heads per core
D2 = HPC * DIM_HEAD  # 128
KT = 4  # k tiles of 128 over QD=512
ISL = 512  # i slice
NI = N // ISL  # 8
JTL = 128  # j tile
NJ = N // JTL  # 32
LAG = 4  # AV matmuls trail QK/exp by this many j-tiles
TOT = NI * NJ  # 256

F32 = mybir.dt.float32
BF16 = mybir.dt.bfloat16
BFNP = ml_dtypes.bfloat16
EXP = mybir.ActivationFunctionType.Exp

# exp staging pattern within each octet of j-tiles (slices >= 1): pos 0-3
# copied PSUM->SBUF by the DVE, exp'd in one [128,4096] ACTIVATE; pos 4-7
# exp'd straight from PSUM.  (GPSIMD cannot access PSUM, so the DVE is the
# only spare stager; 4/8 staged keeps it ~7us/slice under the ACT pace.)


def build_program():
    nc = bacc.Bacc("TRN2", target_bir_lowering=False, debug=False,
                   num_devices=NCORES)

    xT = nc.dram_tensor("xT", [QD, N], BF16, kind="ExternalInput").ap()
    wq = nc.dram_tensor("wq", [QD, D2], BF16, kind="ExternalInput").ap()
    wk = nc.dram_tensor("wk", [QD, D2], BF16, kind="ExternalInput").ap()
    wv = nc.dram_tensor("wv", [QD, D2], BF16, kind="ExternalInput").ap()
    wo = nc.dram_tensor("wo", [D2, QD], BF16, kind="ExternalInput").ap()
    bsel_d = nc.dram_tensor("bsel", [8, 4 * 128], BF16, kind="ExternalInput").ap()
    out = nc.dram_tensor("out", [N, QD], F32, kind="ExternalOutput").ap()

    with TileContext(nc) as tc:
        with tc.tile_pool(name="persist", bufs=1) as pp, \
             tc.tile_pool(name="st_ps", bufs=2, space="PSUM") as st_ps, \
             tc.tile_pool(name="av_ps", bufs=1, space="PSUM") as av_ps, \
             tc.tile_pool(name="aux_ps", bufs=1, space="PSUM") as aux_ps, \
             tc.tile_pool(name="p0_sb", bufs=8) as p0_sb, \
             tc.tile_pool(name="n_sb", bufs=2) as n_sb:
            x_sb = pp.tile([128, KT, N], BF16)
            wq_sb = pp.tile([128, KT, D2], BF16)
            wk_sb = pp.tile([128, KT, D2], BF16)
            wv_sb = pp.tile([128, KT, D2], BF16)
            wo_sb = pp.tile([128, QD], BF16)
            ident = pp.tile([128, 128], BF16)
            identF = pp.tile([128, 128], F32)
            qT = pp.tile([128, N], BF16)
            kT = pp.tile([128, N], BF16)
            vT = pp.tile([128, N], BF16)
            v0p = pp.tile([128, NJ, DIM_HEAD + 1], BF16)
            v1p = pp.tile([128, NJ, DIM_HEAD + 1], BF16)
            ones_col = pp.tile([128, 1], F32)
            ones_bf = pp.tile([128, 1], BF16)
            bsel = pp.tile([8, 4 * 128], BF16)
            nc.sync.dma_start(out=bsel[:], in_=bsel_d[:])

            xTr = xT.rearrange("(k p) n -> p k n", p=128)
            # x slice 0 + wk + wq gate the first QK^T: issue them first, with
            # x slice 0 split into its four k-chunks so the k-projection's
            # accumulation matmuls start as each chunk lands.
            nc.sync.dma_start(out=wk_sb[:], in_=wk.rearrange("(k p) m -> p k m", p=128))
            nc.sync.dma_start(out=x_sb[:, 0, 0:ISL], in_=xTr[:, 0, 0:ISL])
            nc.sync.dma_start(out=x_sb[:, 1, 0:ISL], in_=xTr[:, 1, 0:ISL])
            nc.sync.dma_start(out=wq_sb[:], in_=wq.rearrange("(k p) m -> p k m", p=128))
            nc.sync.dma_start(out=x_sb[:, 2, 0:ISL], in_=xTr[:, 2, 0:ISL])
            nc.sync.dma_start(out=x_sb[:, 3, 0:ISL], in_=xTr[:, 3, 0:ISL])
            nc.sync.dma_start(out=wv_sb[:], in_=wv.rearrange("(k p) m -> p k m", p=128))
            for s in range(1, NI):
                ssl = slice(s * ISL, (s + 1) * ISL)
                nc.sync.dma_start(out=x_sb[:, :, ssl], in_=xTr[:, :, ssl])
            nc.sync.dma_start(out=wo_sb[:], in_=wo[:])
            make_identity(nc, ident[:])
            make_identity(nc, identF[:])
            nc.gpsimd.memset(v0p[:, :, DIM_HEAD], 1.0)
            nc.gpsimd.memset(v1p[:, :, DIM_HEAD], 1.0)
            nc.gpsimd.memset(ones_col[:], 1.0)
            nc.gpsimd.memset(ones_bf[:], 1.0)

            proj_ps = {}

            def proj(w_sb, dst, s, tag="ps", k0=0, k1=KT):
                """dst[:, s*ISL:(s+1)*ISL] = (W^T @ x^T) slice via psum.
                k0/k1 select contraction chunks so the PE work can be spread
                over several emission slots (cast emitted with the last)."""
                ssl = slice(s * ISL, (s + 1) * ISL)
                if k0 == 0:
                    proj_ps[id(dst)] = aux_ps.tile([128, ISL], F32, tag=tag, name="projps") \
                        if tag != "st" else st_ps.tile([128, ISL], F32, tag="st", name="projst")
                ps = proj_ps[id(dst)]
                for k in range(k0, k1):
                    nc.tensor.matmul(ps[:], w_sb[:, k, :], x_sb[:, k, ssl],
                                     start=(k == 0), stop=(k == KT - 1))
                if k1 == KT:
                    nc.vector.tensor_copy(out=dst[:, ssl], in_=ps[:])

            def transp(j):
                """V'[j] tiles from vT via PE transpose (both heads)."""
                tp = aux_ps.tile([128, 128], BF16, tag="aux", name="tp")
                nc.tensor.transpose(tp[:], vT[:, j * JTL:(j + 1) * JTL], ident[:])
                nc.vector.tensor_copy(out=v0p[:, j, 0:DIM_HEAD], in_=tp[:, 0:DIM_HEAD])
                nc.vector.tensor_copy(out=v1p[:, j, 0:DIM_HEAD], in_=tp[:, DIM_HEAD:D2])

            states = {}

            def epi(i_prev, step, tail=False):
                """Deferred epilogue for slice i_prev (runs during i_prev+1)."""
                e = states[i_prev]
                if step == 0:  # av -> SBUF (frees av PSUM; source for den/lh)
                    e["av_sb0"] = n_sb.tile([DIM_HEAD + 1, ISL], F32, tag="av_sb0", name="av_sb0")
                    e["av_sb1"] = n_sb.tile([DIM_HEAD + 1, ISL], F32, tag="av_sb1", name="av_sb1")
                    nc.vector.tensor_copy(out=e["av_sb0"][:], in_=e["av0"][:])
                    nc.vector.tensor_copy(out=e["av_sb1"][:], in_=e["av1"][:])
                elif step == 1:  # den rows to bf16 (one partition each)
                    e["db0"] = n_sb.tile([1, ISL], BF16, tag="db0", name="db0")
                    e["db1"] = n_sb.tile([1, ISL], BF16, tag="db1", name="db1")
                    nc.vector.tensor_copy(out=e["db0"][:],
                                          in_=e["av_sb0"][DIM_HEAD:DIM_HEAD + 1, :])
                    nc.vector.tensor_copy(out=e["db1"][:],
                                          in_=e["av_sb1"][DIM_HEAD:DIM_HEAD + 1, :])
                elif step == 11:  # den rows -> partitions via eight K=1 matmuls
                    e["dnt"] = aux_ps.tile([128, 8], F32, tag="ps", name="dnt")
                    for s in range(4):
                        for h in range(2):
                            db = e["db0"] if h == 0 else e["db1"]
                            c = 2 * s + h
                            nc.tensor.matmul(
                                e["dnt"][:, c:c + 1],
                                db[0:1, s * 128:(s + 1) * 128],
                                ones_bf[0:1, 0:1],
                                start=True, stop=True)
                elif step == 2:  # one wide reciprocal (0.2us vs 2x 3.3us)
                    e["rT"] = n_sb.tile([128, 8], F32, tag="rT", name="rT")
                    nc.vector.reciprocal(e["rT"][:], e["dnt"][:])
                elif step == 3:  # transpose rT back: [8, 128] = rT^T
                    e["rtt_ps"] = aux_ps.tile([8, 128], F32, tag="ps", name="rtt_ps")
                    nc.tensor.matmul(e["rtt_ps"][:], e["rT"][:], identF[:],
                                     start=True, stop=True)
                elif step == 4:
                    e["rtt"] = n_sb.tile([8, 128], BF16, tag="rtt", name="rtt")
                    nc.vector.tensor_copy(out=e["rtt"][:], in_=e["rtt_ps"][:])
                elif step == 5:  # broadcast recips along d2 via 4 PE matmuls
                    e["rb"] = aux_ps.tile([128, ISL], F32, tag="aux", name="rb")
                    for s in range(4):
                        nc.tensor.matmul(e["rb"][:, s * 128:(s + 1) * 128],
                                         bsel[:, s * 128:(s + 1) * 128],
                                         e["rtt"][:], start=True, stop=True)
                elif step == 6:  # normalize -> lh (bf16), column-split so
                    # the first Wo matmuls only wait for lh cols 0..255
                    e["lh"] = n_sb.tile([128, ISL], BF16, tag="lh", name="lh")
                    for cs in (slice(0, 256), slice(256, 512)):
                        nc.vector.tensor_mul(out=e["lh"][0:64, cs],
                                             in0=e["av_sb0"][0:DIM_HEAD, cs],
                                             in1=e["rb"][0:64, cs])
                        nc.vector.tensor_mul(out=e["lh"][64:128, cs],
                                             in0=e["av_sb1"][0:DIM_HEAD, cs],
                                             in1=e["rb"][64:128, cs])
                else:  # steps 7..10: one Wo matmul + store each
                    s = step - 7
                    if tail:  # st pool is idle at the tail: wider ladder
                        wop = st_ps.tile([128, QD], F32, tag="st", name=f"wot{s}")
                    else:
                        wop = aux_ps.tile([128, QD], F32, tag="aux", name="wop")
                    nc.tensor.matmul(wop[:], e["lh"][:, s * 128:(s + 1) * 128],
                                     wo_sb[:], start=True, stop=True)
                    wos = n_sb.tile([128, QD], F32, tag="wos", name="wos", bufs=4)
                    nc.vector.tensor_copy(out=wos[:], in_=wop[:])
                    deng = nc.scalar if (tail and s % 2) else nc.sync
                    deng.dma_start(
                        out=out[i_prev * ISL + s * 128:i_prev * ISL + (s + 1) * 128, :],
                        in_=wos[:])

            # epilogue emission slots (j within the following slice)
            EPI = {4: 0, 7: 1, 8: 11, 9: 2, 12: 3, 13: 4, 14: 5, 15: 6,
                   16: 7, 18: 8, 20: 9, 22: 10}

            # virtual-clock pacing (see module docstring)
            HEAD_US = 13.0
            PACE0_US = 1.65
            PACE_US = 1.16

            def slot_ms(g):
                if g < NJ:
                    return (HEAD_US + g * PACE0_US) / 1e3
                return (HEAD_US + NJ * PACE0_US + (g - NJ) * PACE_US) / 1e3

            # warm-up: k and q projections for slice 0 (independent st slots)
            proj(wk_sb, kT, 0, tag="st")
            proj(wq_sb, qT, 0, tag="st")

            octs = {}  # (i, o) -> dict with sga/sgb/pta/ptb/ptu tiles

            def p_src(ga):
                """(ap, col0) holding exp'd tile ga for the AV matmuls."""
                return octs[ga], 0

            for g in range(TOT + LAG):
                tc.tile_set_cur_wait(ms=slot_ms(g))
                i, j = divmod(g, NJ) if g < TOT else (NI, g - TOT)
                if g < TOT:
                    if j == 0:
                        states[i] = {
                            "av0": av_ps.tile([DIM_HEAD + 1, ISL], F32, tag="av0", name="av0"),
                            "av1": av_ps.tile([DIM_HEAD + 1, ISL], F32, tag="av1", name="av1"),
                        }
                    isl = slice(i * ISL, (i + 1) * ISL)
                    jsl = slice(j * JTL, (j + 1) * JTL)
                    st = st_ps.tile([128, 2 * ISL], F32, tag="st", name="st")
                    nc.tensor.matmul(st[:, 0:ISL], kT[0:64, jsl], qT[0:64, isl],
                                     start=True, stop=True)
                    nc.tensor.matmul(st[:, ISL:2 * ISL], kT[64:128, jsl],
                                     qT[64:128, isl], start=True, stop=True)
                    pt = p0_sb.tile([128, 2 * ISL], BF16, tag="pt0", name="pt0")
                    nc.scalar.activation(pt[:], st[:], EXP, scale=SCALE)
                    octs[g] = pt
                if g >= LAG:
                    ga = g - LAG
                    ia, ja = divmod(ga, NJ)
                    src, c0 = p_src(ga)
                    eia = states[ia]
                    nc.tensor.matmul(eia["av0"][:], v0p[:, ja, :],
                                     src[:, c0:c0 + ISL],
                                     start=(ja == 0), stop=(ja == NJ - 1))
                    nc.tensor.matmul(eia["av1"][:], v1p[:, ja, :],
                                     src[:, c0 + ISL:c0 + 2 * ISL],
                                     start=(ja == 0), stop=(ja == NJ - 1))
                # slice-0 prologue: stream k/v/V' production
                if i == 0:
                    if j == 0:
                        proj(wv_sb, vT, 0)
                    elif j == 1:
                        proj(wk_sb, kT, 1)
                    elif j == 2:
                        proj(wv_sb, vT, 1)
                    elif j == 3:
                        for jj in range(4):
                            transp(jj)
                    elif j == 4:
                        for jj in range(4, 8):
                            transp(jj)
                    elif j >= 5:
                        s = (j - 1) // 4 + 1
                        m = (j - 1) % 4
                        if s < NI:
                            if m == 0:
                                proj(wk_sb, kT, s, k0=0, k1=2)
                            elif m == 1:
                                proj(wk_sb, kT, s, k0=2, k1=4)
                                proj(wv_sb, vT, s, k0=0, k1=2)
                            elif m == 2:
                                proj(wv_sb, vT, s, k0=2, k1=4)
                                transp(4 * s)
                                transp(4 * s + 1)
                            else:
                                transp(4 * s + 2)
                                transp(4 * s + 3)
                # deferred epilogue of slice i-1
                if 1 <= i < NI and j in EPI:
                    epi(i - 1, EPI[j])
                # next slice's q projection
                if g < TOT and j == 10 and i + 1 < NI:
                    proj(wq_sb, qT, i + 1)

            # tail: full epilogue for the last slice
            for sidx, step in enumerate((0, 1, 11, 2, 3, 4, 5, 6, 7, 8, 9, 10)):
                tc.tile_set_cur_wait(ms=slot_ms(TOT + LAG) + 0.0002 * sidx)
                epi(NI - 1, step, tail=True)

    nc.compile()
    return nc


_NC = None


def _get_program():
    global _NC
    if _NC is None:
        _NC = build_program()
    return _NC


def _bsel_host():
    b = np.zeros((8, 512), dtype=np.float32)
    for s in range(4):
        b[2 * s, s * 128:s * 128 + 64] = 1.0
        b[2 * s + 1, s * 128 + 64:(s + 1) * 128] = 1.0
    return b.astype(BFNP)


def kernel(x, Wq, Wk, Wv, Wo, bo):
    x = np.asarray(x, dtype=np.float32)
    Wq = np.asarray(Wq, dtype=np.float32)
    Wk = np.asarray(Wk, dtype=np.float32)
    Wv = np.asarray(Wv, dtype=np.float32)
    Wo = np.asarray(Wo, dtype=np.float32)
    bo = np.asarray(bo, dtype=np.float32)

    nc = _get_program()

    in_maps = []
    for c in range(NCORES):
        b, m = divmod(c, NCORES // B)
        cs = slice(m * D2, (m + 1) * D2)
        in_maps.append({
            "xT": np.ascontiguousarray(x[b].T).astype(BFNP),
            "wq": np.ascontiguousarray(Wq[:, cs]).astype(BFNP),
            "wk": np.ascontiguousarray(Wk[:, cs]).astype(BFNP),
            "wv": np.ascontiguousarray(Wv[:, cs]).astype(BFNP),
            "wo": np.ascontiguousarray(Wo[cs, :]).astype(BFNP),
            "bsel": _bsel_host(),
        })

    res = run_bass_kernel_spmd(nc, in_maps, core_ids=list(range(NCORES)))

    out = np.zeros((B, N, QD), dtype=np.float32)
    for c in range(NCORES):
        b = c // (NCORES // B)
        out[b] += res.results[c]["out"]
    out += bo[None, None, :]
    return out
